# revision 1
# baseline (speedup 1.0000x reference)
"""CaptioningRNN forward loss on 8 Trainium2 NeuronCores.

Math (per reference):
    h0 = features @ W_proj + b_proj                       (no tanh)
    x  = W_embed[captions[:, :-1]]
    a  = x @ Wx + b                                       (precomputed input drive)
    h_t = tanh(h_{t-1} @ Wh + a_t)                        (T sequential steps)
    s  = h @ W_out + b_out                                (N*T x V logits)
    loss = sum over (n,t) of mask * (logsumexp(s) - s[target]) / N

Sharding: data-parallel over batch N=256 -> 32 rows/core, weights replicated.
Each core returns a partial masked-NLL sum; host adds the 8 scalars and
divides by N.

On-chip strategy (per core, all t-major with rows r = t*32 + n):
  * tokens gathered from W_embed via indirect DMA, transposed to xT with the
    DMA xbar, a = Wx.T-form matmul -> aT (bf16)
  * recurrence in transposed form: hT[:, t] = tanh(Wh-blocks @ hT[:, t-1] + aT)
    one [128,128] PSUM tile per step (4 h'-chunks side by side in free dim)
  * logits never materialized: for each 128-row tile and each 1024-wide vocab
    group, matmul into PSUM and one ScalarE Exp with accum_out produces the
    partial row-sum of exp directly; logsumexp = Ln(sum of partials).
  * target score: rows of W_out.T (augmented with b_out column) gathered by
    target token via indirect DMA; dot with h rows via one fused
    tensor_tensor_reduce per tile.
  * bias b / b_proj applied via per-partition activation bias (T-form layout
    puts the hidden dim on partitions).  b_out enters through the augmented
    gather column (and is zero in this problem; see note in _build()).
"""

import sys

for _p in ("/opt/trn_rl_repo", "/root/.axon_site/_ro/trn_rl_repo"):
    if _p not in sys.path:
        sys.path.insert(0, _p)

import numpy as np
import ml_dtypes
from contextlib import ExitStack

import concourse.bass as bass
import concourse.tile as tile
from concourse import bacc, mybir
from concourse.bass import IndirectOffsetOnAxis
from concourse.bass_utils import run_bass_kernel_spmd

F32 = mybir.dt.float32
BF16 = mybir.dt.bfloat16
FP8 = mybir.dt.float8e4
I32 = mybir.dt.int32
AF = mybir.ActivationFunctionType
OP = mybir.AluOpType

# Problem sizes (hardcoded per spec).
N, T, D, W, H, V = 256, 64, 1280, 256, 512, 10000
NCORES = 8
NS = N // NCORES          # 32 batch rows per core
R = NS * T                # 2048 (t-major rows per core)
MT = R // 128             # 16 row tiles
KH = H // 128             # 4 hidden chunks
KW = W // 128             # 2 embed chunks
KD = D // 128             # 10 feature chunks
TSLOT = T + 1             # h slots (0 = h0)
HTB = TSLOT * NS          # 2080 columns per hidden-chunk block of hT
AUG = 514                 # gathered W_out^T row: 512 + b_out + pad
P = 128

# vocab tiling: 512-wide matmuls, paired into <=1024-wide exp groups
_VT = []
v = 0
while v < V:
    w = min(512, V - v)
    _VT.append((v, w))
    v += w
VGROUPS = []          # list of list[(voff, width)]
i = 0
while i < len(_VT):
    VGROUPS.append(_VT[i:i + 3])
    i += 3
NG = len(VGROUPS)     # 7 (6x1536 + 800)

_CACHE = {}
_DEBUG = False
_ABL_NO_SCORES = False
_ABL_NO_REC = False
_ABL_NO_ST = False
_ABL_NO_PRE = False
_ABL_NO_GATHER = False
_ABL_NO_FINAL = False
_WOUT_SPLIT = 2500  # DMA chunk width for W_out load (V = one DMA per k)


def _build(with_bout_mm: bool, zero_bias: bool = True):
    """Build + compile the per-core Bass program (identical across cores)."""
    nc = bacc.Bacc(
        "TRN2", target_bir_lowering=False, debug=False, num_devices=NCORES
    )

    featT = nc.dram_tensor("featT", [D, NS], BF16, kind="ExternalInput")
    tok_in = nc.dram_tensor("tok_in", [P, MT], I32, kind="ExternalInput")
    tok_out = nc.dram_tensor("tok_out", [P, MT], I32, kind="ExternalInput")
    w_out = nc.dram_tensor("w_out", [H, V], FP8, kind="ExternalInput")
    w_outT = nc.dram_tensor("w_outT", [V, AUG], BF16, kind="ExternalInput")
    w_embed = nc.dram_tensor("w_embed", [V, W], BF16, kind="ExternalInput")
    wh_d = nc.dram_tensor("wh", [H, H], BF16, kind="ExternalInput")
    wx_d = nc.dram_tensor("wx", [W, H], BF16, kind="ExternalInput")
    wproj_d = nc.dram_tensor("wproj", [D, H], BF16, kind="ExternalInput")
    b_d = nc.dram_tensor("b", [H, 1], F32, kind="ExternalInput")
    bp_d = nc.dram_tensor("b_proj", [H, 1], F32, kind="ExternalInput")
    bo_d = nc.dram_tensor("b_out_row", [1, V], F32, kind="ExternalInput")
    loss_d = nc.dram_tensor("loss", [1, 1], F32, kind="ExternalOutput")
    scratch_d = nc.dram_tensor("scratch", [P, 1], F32)  # internal
    if _DEBUG:
        dbg_partials = nc.dram_tensor("dbg_partials", [P, MT * NG], F32,
                                      kind="ExternalOutput")
        dbg_st = nc.dram_tensor("dbg_st", [P, MT], F32, kind="ExternalOutput")
        dbg_acc = nc.dram_tensor("dbg_acc", [P, MT], F32, kind="ExternalOutput")
        dbg_lse = nc.dram_tensor("dbg_lse", [P, MT], F32, kind="ExternalOutput")
        dbg_tot = nc.dram_tensor("dbg_tot", [P, 1], F32, kind="ExternalOutput")
        dbg_h = nc.dram_tensor("dbg_h", [P, MT * H], BF16, kind="ExternalOutput")

    with tile.TileContext(nc) as tc, ExitStack() as ctx:
        const = ctx.enter_context(tc.tile_pool(name="const", bufs=1))
        work = ctx.enter_context(tc.tile_pool(name="work", bufs=3))
        psum_sc = ctx.enter_context(tc.tile_pool(name="psc", bufs=2, space="PSUM"))
        psum_st = ctx.enter_context(tc.tile_pool(name="pst", bufs=1, space="PSUM"))
        psum_ms = ctx.enter_context(tc.tile_pool(name="pms", bufs=1, space="PSUM"))

        # ---- persistent SBUF tensors ----
        wout_sb = const.tile([P, KH * V], FP8)       # 40KB/part
        hT8 = const.tile([P, KH * R], FP8)           # fp8 copy of hT slots 1..64
        hT = const.tile([P, KH * HTB], BF16)         # 16.6KB/part
        aT = const.tile([P, KH * R], BF16)           # 16.4KB/part (x @ Wx + b)^T
        wg_sb = const.tile([P, MT * AUG], BF16)      # gathered target W_out rows
        h_rows = const.tile([P, MT * H], BF16)       # h row-major (DMA-transposed)
        xT_sb = const.tile([P, KW * R], BF16)
        wh_sb = const.tile([P, KH * KH * P], BF16)
        wx_sb = const.tile([P, KW * KH * P], BF16)
        tok_in_sb = const.tile([P, MT], I32)
        tok_out_sb = const.tile([P, MT], I32)
        b_sb = const.tile([P, KH], F32)
        bp_sb = const.tile([P, KH], F32)
        partials = const.tile([P, MT * NG], F32)
        st_all = const.tile([P, MT], F32)
        acc = const.tile([P, MT], F32)
        lse = const.tile([P, MT], F32)
        nll = const.tile([P, MT], F32)
        mask = const.tile([P, MT], F32)
        tot = const.tile([P, 1], F32)
        tot_row = const.tile([1, P], F32)
        res = const.tile([1, 1], F32)
        warm = const.tile([P, 1], F32)
        if with_bout_mm:
            bo_sb = const.tile([1, V], F32)
            onesr = const.tile([1, P], F32)

        # ---- token / small-weight loads first: they gate the front-end
        # compute chain (gathers -> xT -> a -> recurrence).  W_out (10.2MB,
        # needed only once scores start) goes last, on the ScalarE HWDGE
        # queue so it doesn't head-of-line-block the SP queue.
        nc.sync.dma_start(tok_in_sb[:], tok_in[:, :])
        nc.sync.dma_start(tok_out_sb[:], tok_out[:, :])
        # biases: column k holds bias chunk for hidden block k
        nc.sync.dma_start(b_sb[:], bass.AP(b_d, 0, [[1, P], [P, KH]]))
        nc.sync.dma_start(bp_sb[:], bass.AP(bp_d, 0, [[1, P], [P, KH]]))
        # block layouts (k*KH+mp)*P are contiguous per k-chunk, so one DMA
        # per k row-slab; wproj/featT first (they gate h0 = PE's first work)
        wproj_sb = const.tile([P, KD * KH * P], BF16)
        featT_sb = const.tile([P, KD * NS], BF16)
        for k in range(KD):
            nc.sync.dma_start(wproj_sb[:, k * H:(k + 1) * H],
                              wproj_d[k * P:(k + 1) * P, :])
            nc.sync.dma_start(featT_sb[:, k * NS:(k + 1) * NS],
                              featT[k * P:(k + 1) * P, :])
        for k in range(KH):
            nc.sync.dma_start(wh_sb[:, k * H:(k + 1) * H],
                              wh_d[k * P:(k + 1) * P, :])
        for k in range(KW):
            nc.sync.dma_start(wx_sb[:, k * H:(k + 1) * H],
                              wx_d[k * P:(k + 1) * P, :])
        v0 = 0
        while v0 < V:
            wd = min(_WOUT_SPLIT, V - v0)
            for k in range(KH):
                nc.sync.dma_start(wout_sb[:, k * V + v0:k * V + v0 + wd],
                                  w_out[k * P:(k + 1) * P, v0:v0 + wd])
            v0 += wd
        if with_bout_mm:
            nc.sync.dma_start(bo_sb[:], bo_d[:, :])
            nc.gpsimd.memset(onesr[:], 1.0)

        # warm the ACT exp/tanh table set early (overlaps the big DMAs)
        nc.gpsimd.memset(warm[:], 0.0)
        nc.scalar.activation(warm[:], warm[:], AF.Exp)
        if _ABL_NO_SCORES:
            nc.gpsimd.memset(partials[:], 1.0)
        if _ABL_NO_ST:
            nc.gpsimd.memset(st_all[:], 0.0)
        if _ABL_NO_REC:
            nc.gpsimd.memset(hT[:], 0.0)

        ident = const.tile([P, P], BF16)
        from concourse.masks import make_identity
        make_identity(nc, ident[:])

        # ---- h0 first: PE's earliest work, gated only on wproj/featT ----
        if not _ABL_NO_PRE:
            for mp in range(KH):
                ps = psum_ms.tile([P, NS], F32, tag="ms")
                for k in range(KD):
                    nc.tensor.matmul(ps[:], lhsT=wproj_sb[:, (k * KH + mp) * P:
                                                          (k * KH + mp + 1) * P],
                                     rhs=featT_sb[:, k * NS:(k + 1) * NS],
                                     start=(k == 0), stop=(k == KD - 1))
                if zero_bias:
                    nc.vector.tensor_copy(hT[:, mp * HTB:mp * HTB + NS], ps[:])
                else:
                    nc.scalar.activation(hT[:, mp * HTB:mp * HTB + NS], ps[:],
                                         AF.Identity, bias=bp_sb[:, mp:mp + 1])

        # ---- gathers; xT via PE identity-transpose (PE is idle here and the
        # SP DMA queue would serialize 32 xbar transposes head-of-line).
        # Emitted per 512-row chunk, fused into the recurrence loop below:
        # chunk mc feeds exactly rec steps 4mc+1..4mc+4.
        xg_tiles = []
        if not _ABL_NO_GATHER:
            for mc in range(4):
                xg = const.tile([P, 4 * W], BF16, name=f"xg{mc}")
                xg_tiles.append(xg)
                for j in range(4):
                    m = mc * 4 + j
                    nc.gpsimd.indirect_dma_start(
                        out=xg[:, j * W:(j + 1) * W],
                        out_offset=None,
                        in_=w_embed[:, :],
                        in_offset=IndirectOffsetOnAxis(
                            ap=tok_in_sb[:, m:m + 1], axis=0),
                    )

        def emit_gather_chunk(mc):
            xg = xg_tiles[mc]
            for j in range(4):
                m = mc * 4 + j
                for k in range(KW):
                    # early in the kernel the scores PSUM pool is idle; use
                    # its double-buffered slots so transposes pipeline
                    pst_x = psum_sc.tile([P, P], BF16, tag="sc", name="pst_x")
                    nc.tensor.transpose(
                        pst_x[:], xg[:, j * W + k * P:j * W + (k + 1) * P],
                        ident[:])
                    nc.vector.tensor_copy(
                        xT_sb[:, k * R + m * P:k * R + (m + 1) * P], pst_x[:])
            # aT rows for this 512-row chunk immediately (rec step 1 needs
            # only chunk 0 -- don't make it wait for all 16 gathers)
            if not _ABL_NO_PRE:
                nch = mc
                for mp in range(KH):
                    ps = psum_ms.tile([P, 512], F32, tag="ms")
                    for k in range(KW):
                        nc.tensor.matmul(
                            ps[:],
                            lhsT=wx_sb[:, (k * KH + mp) * P:(k * KH + mp + 1) * P],
                            rhs=xT_sb[:, k * R + nch * 512:k * R + (nch + 1) * 512],
                            start=(k == 0), stop=(k == KW - 1))
                    if zero_bias:
                        nc.vector.tensor_copy(
                            aT[:, mp * R + nch * 512:mp * R + (nch + 1) * 512],
                            ps[:])
                    else:
                        nc.scalar.activation(
                            aT[:, mp * R + nch * 512:mp * R + (nch + 1) * 512],
                            ps[:], AF.Identity, bias=b_sb[:, mp:mp + 1])

        # target-column rows of W_out^T (+ b_out in col 512).  Emitted per-m
        # inside the main loop AFTER that chunk's x-gathers so the Pool queue
        # serves the ramp-critical x path first.
        def emit_wg_gather(m):
            nc.gpsimd.indirect_dma_start(
                out=wg_sb[:, m * AUG:(m + 1) * AUG],
                out_offset=None,
                in_=w_outT[:, :],
                in_offset=IndirectOffsetOnAxis(ap=tok_out_sb[:, m:m + 1], axis=0),
            )

        hT3 = hT[:].rearrange("p (b c) -> p b c", b=KH)
        aT3 = aT[:].rearrange("p (b c) -> p b c", b=KH)
        hT8_3 = hT8[:].rearrange("p (b c) -> p b c", b=KH)
        wout3 = wout_sb[:].rearrange("p (k c) -> p k c", k=KH)

        def emit_step(t):
            """h slot t (1..T) from slot t-1; PSUM [128, 4*NS], blocks = h'-chunks.
            The drive a_t enters via an identity matmul that opens the
            accumulation group, keeping the whole step chain on PE + ACT."""
            ps = psum_st.tile([P, KH * NS], F32, tag="step")
            nc.tensor.matmul(
                ps[:], lhsT=ident[:],
                rhs=aT3[:, :, (t - 1) * NS:t * NS],
                start=True, stop=False, skip_group_check=True)
            for mp in range(KH):
                for k in range(KH):
                    nc.tensor.matmul(
                        ps[:, mp * NS:(mp + 1) * NS],
                        lhsT=wh_sb[:, (k * KH + mp) * P:(k * KH + mp + 1) * P],
                        rhs=hT[:, k * HTB + (t - 1) * NS:k * HTB + t * NS],
                        start=False, stop=(mp == KH - 1 and k == KH - 1),
                        skip_group_check=True)
            ps3 = ps[:].rearrange("p (b n) -> p b n", b=KH)
            nc.scalar.activation(hT3[:, :, t * NS:(t + 1) * NS], ps3[:], AF.Tanh)
            # fp8 shadow of h_t for the DoubleRow score matmuls
            nc.vector.tensor_copy(hT8_3[:, :, (t - 1) * NS:t * NS],
                                  hT3[:, :, t * NS:(t + 1) * NS])

        def emit_scores(m, groups):
            """vocab exp-sums for row tile m over the given vocab groups."""
            for gi in groups:
                ps = psum_sc.tile([P, 1536], F32, tag="sc")
                off = 0
                for (voff, wd) in VGROUPS[gi]:
                    for g in range(KH // 2):
                        nc.tensor.matmul(
                            ps[:, off:off + wd],
                            lhsT=hT8_3[:, 2 * g:2 * g + 2, m * P:(m + 1) * P],
                            rhs=wout3[:, 2 * g:2 * g + 2, voff:voff + wd],
                            start=(g == 0),
                            stop=(g == KH // 2 - 1 and not with_bout_mm),
                            perf_mode=mybir.MatmulPerfMode.DoubleRow)
                    if with_bout_mm:
                        nc.tensor.matmul(
                            ps[:, off:off + wd],
                            lhsT=onesr[:, :],
                            rhs=bo_sb[:, voff:voff + wd],
                            start=False, stop=True,
                            skip_group_check=True)
                    off += wd
                esc = work.tile([P, 1536], BF16, tag="esc")
                nc.scalar.activation(esc[:, :off], ps[:, :off], AF.Exp,
                                     accum_out=partials[:, m * NG + gi:m * NG + gi + 1])

        def emit_hrows_st(m):
            for k in range(KH):
                nc.sync.dma_start_transpose(
                    h_rows[:, m * H + k * P:m * H + (k + 1) * P],
                    hT[:, k * HTB + NS + m * P:k * HTB + NS + (m + 1) * P])
            junk = work.tile([P, H], BF16, tag="junk")
            nc.vector.tensor_mul(junk[:], h_rows[:, m * H:(m + 1) * H],
                                 wg_sb[:, m * AUG:m * AUG + H])
            stp = work.tile([P, 1], F32, tag="stp")
            nc.vector.tensor_reduce(stp[:], junk[:],
                                    axis=mybir.AxisListType.X, op=OP.add)
            # + b_out[target] from the augmented gather column (zero here)
            nc.vector.tensor_add(st_all[:, m:m + 1], stp[:],
                                 wg_sb[:, m * AUG + H:m * AUG + H + 1])

        # ---- recurrence interleaved with scores of the previous row tile;
        # the first four iterations also run the gather->xT->aT pipeline ----
        group_chunks = [list(range(0, 2)), list(range(2, 4)),
                        list(range(4, 6)), list(range(6, NG))]
        for m in range(MT):
            if m < 4 and not _ABL_NO_GATHER:
                emit_gather_chunk(m)
            if not _ABL_NO_GATHER:
                emit_wg_gather(m)
            for j in range(4):
                if not _ABL_NO_REC:
                    emit_step(4 * m + j + 1)
                if m >= 1 and not _ABL_NO_SCORES:
                    emit_scores(m - 1, group_chunks[j])
            if not _ABL_NO_ST:
                emit_hrows_st(m)
        if not _ABL_NO_SCORES:
            emit_scores(MT - 1, list(range(NG)))

        # ---- loss assembly ----
        pr3 = partials[:].rearrange("p (m g) -> p m g", m=MT)
        nc.vector.tensor_reduce(acc[:], pr3[:], axis=mybir.AxisListType.X,
                                op=OP.add)
        nc.scalar.activation(lse[:], acc[:], AF.Ln)
        nc.vector.tensor_sub(nll[:], lse[:], st_all[:])
        nc.vector.tensor_scalar(mask[:], tok_out_sb[:], 0, None,
                                op0=OP.not_equal)
        junk2 = const.tile([P, MT], F32)
        nc.vector.tensor_mul(junk2[:], nll[:], mask[:])
        nc.vector.tensor_reduce(tot[:], junk2[:],
                                axis=mybir.AxisListType.X, op=OP.add)
        # cross-partition sum: bounce [128,1] -> DRAM -> [1,128], reduce
        nc.sync.dma_start(scratch_d[:, :], tot[:])
        nc.sync.dma_start(tot_row[:], bass.AP(scratch_d, 0, [[P, 1], [1, P]]))
        nc.vector.tensor_reduce(res[:], tot_row[:], axis=mybir.AxisListType.X,
                                op=OP.add)
        nc.sync.dma_start(loss_d[:, :], res[:])
        if _DEBUG:
            nc.sync.dma_start(dbg_partials[:, :], partials[:])
            nc.sync.dma_start(dbg_st[:, :], st_all[:])
            nc.sync.dma_start(dbg_acc[:, :], acc[:])
            nc.sync.dma_start(dbg_lse[:, :], lse[:])
            nc.sync.dma_start(dbg_tot[:, :], tot[:])
            nc.sync.dma_start(dbg_h[:, :], h_rows[:])

    nc.compile()
    return nc


def _prepare_inputs(inputs):
    """Cast/shard host-side. Returns per-core in_maps."""
    feats = np.asarray(inputs["features"], dtype=np.float32)
    cap = np.asarray(inputs["captions"])
    W_proj = np.asarray(inputs["W_proj"], dtype=np.float32)
    b_proj = np.asarray(inputs["b_proj"], dtype=np.float32).reshape(H, 1)
    W_embed = np.asarray(inputs["W_embed"], dtype=np.float32)
    Wx = np.asarray(inputs["Wx"], dtype=np.float32)
    Wh = np.asarray(inputs["Wh"], dtype=np.float32)
    b = np.asarray(inputs["b"], dtype=np.float32).reshape(H, 1)
    W_out = np.asarray(inputs["W_out"], dtype=np.float32)
    b_out = np.asarray(inputs["b_out"], dtype=np.float32)

    bf = ml_dtypes.bfloat16
    f8 = ml_dtypes.float8_e4m3
    w_out_f8 = np.ascontiguousarray(W_out.astype(f8))
    w_embed_bf = np.ascontiguousarray(W_embed.astype(bf))
    wh_bf = np.ascontiguousarray(Wh.astype(bf))
    wx_bf = np.ascontiguousarray(Wx.astype(bf))
    wproj_bf = np.ascontiguousarray(W_proj.astype(bf))
    w_outT = np.zeros((V, AUG), dtype=bf)
    w_outT[:, :H] = W_out.T.astype(bf)
    w_outT[:, H] = b_out.astype(bf)
    bo_row = np.ascontiguousarray(b_out.reshape(1, V))

    shared = {
        "w_out": w_out_f8, "w_outT": w_outT, "w_embed": w_embed_bf,
        "wh": wh_bf, "wx": wx_bf, "wproj": wproj_bf,
        "b": b, "b_proj": b_proj, "b_out_row": bo_row,
    }
    in_maps = []
    for c in range(NCORES):
        rows = slice(c * NS, (c + 1) * NS)
        featT_c = np.ascontiguousarray(feats[rows].T.astype(bf))
        cin = np.asarray(cap[rows, :T], dtype=np.int32)     # [NS, T]
        cout = np.asarray(cap[rows, 1:T + 1], dtype=np.int32)
        # t-major flat r = t*NS + n, laid out as [128, MT] with r = m*128 + i
        tin = np.ascontiguousarray(cin.T.reshape(R).reshape(MT, P).T)
        tout = np.ascontiguousarray(cout.T.reshape(R).reshape(MT, P).T)
        in_maps.append({**shared, "featT": featT_c,
                        "tok_in": tin, "tok_out": tout})
    zero_bias = not (np.any(b) or np.any(b_proj))
    return in_maps, (bool(np.any(b_out != 0.0)), zero_bias)


def _get_program(flags=(False, True)):
    key = ("nc",) + tuple(flags)
    if key not in _CACHE:
        _CACHE[key] = _build(*flags)
    return _CACHE[key]


def kernel(**inputs) -> np.ndarray:
    in_maps, flags = _prepare_inputs(inputs)
    nc = _get_program(flags)
    out = run_bass_kernel_spmd(nc, in_maps, core_ids=list(range(NCORES)))
    total = sum(float(r["loss"][0, 0]) for r in out.results)
    return np.float32(total / N)



# revision 7
# speedup vs baseline: 1.0172x; 1.0172x over previous
"""CaptioningRNN forward loss on 8 Trainium2 NeuronCores.

Math (per reference):
    h0 = features @ W_proj + b_proj                       (no tanh)
    x  = W_embed[captions[:, :-1]]
    a  = x @ Wx + b                                       (precomputed input drive)
    h_t = tanh(h_{t-1} @ Wh + a_t)                        (T sequential steps)
    s  = h @ W_out + b_out                                (N*T x V logits)
    loss = sum over (n,t) of mask * (logsumexp(s) - s[target]) / N

Sharding: data-parallel over batch N=256 -> 32 rows/core, weights replicated.
Each core returns a partial masked-NLL sum; host adds the 8 scalars and
divides by N.

On-chip strategy (per core, all t-major with rows r = t*32 + n):
  * tokens gathered from W_embed via indirect DMA, transposed to xT with the
    DMA xbar, a = Wx.T-form matmul -> aT (bf16)
  * recurrence in transposed form: hT[:, t] = tanh(Wh-blocks @ hT[:, t-1] + aT)
    one [128,128] PSUM tile per step (4 h'-chunks side by side in free dim)
  * logits never materialized: for each 128-row tile and each 1024-wide vocab
    group, matmul into PSUM and one ScalarE Exp with accum_out produces the
    partial row-sum of exp directly; logsumexp = Ln(sum of partials).
  * target score: rows of W_out.T (augmented with b_out column) gathered by
    target token via indirect DMA; dot with h rows via one fused
    tensor_tensor_reduce per tile.
  * bias b / b_proj applied via per-partition activation bias (T-form layout
    puts the hidden dim on partitions).  b_out enters through the augmented
    gather column (and is zero in this problem; see note in _build()).
"""

import sys

for _p in ("/opt/trn_rl_repo", "/root/.axon_site/_ro/trn_rl_repo"):
    if _p not in sys.path:
        sys.path.insert(0, _p)

import numpy as np
import ml_dtypes
from contextlib import ExitStack

import concourse.bass as bass
import concourse.tile as tile
from concourse import bacc, mybir
from concourse.bass import IndirectOffsetOnAxis
from concourse.bass_utils import run_bass_kernel_spmd

F32 = mybir.dt.float32
BF16 = mybir.dt.bfloat16
FP8 = mybir.dt.float8e4
I32 = mybir.dt.int32
AF = mybir.ActivationFunctionType
OP = mybir.AluOpType

# Problem sizes (hardcoded per spec).
N, T, D, W, H, V = 256, 64, 1280, 256, 512, 10000
NCORES = 8
NS = N // NCORES          # 32 batch rows per core
R = NS * T                # 2048 (t-major rows per core)
MT = R // 128             # 16 row tiles
KH = H // 128             # 4 hidden chunks
KW = W // 128             # 2 embed chunks
KD = D // 128             # 10 feature chunks
TSLOT = T + 1             # h slots (0 = h0)
HTB = TSLOT * NS          # 2080 columns per hidden-chunk block of hT
AUG = 514                 # gathered W_out^T row: 512 + b_out + pad
P = 128

# vocab tiling: 512-wide matmuls, paired into <=1024-wide exp groups
_VT = []
v = 0
while v < V:
    w = min(512, V - v)
    _VT.append((v, w))
    v += w
VGROUPS = []          # list of list[(voff, width)]
i = 0
while i < len(_VT):
    VGROUPS.append(_VT[i:i + 3])
    i += 3
NG = len(VGROUPS)     # 7 (6x1536 + 800)

_CACHE = {}
_DEBUG = False
_ABL_NO_SCORES = False
_ABL_NO_REC = False
_ABL_NO_ST = False
_ABL_NO_PRE = False
_ABL_NO_GATHER = False
_ABL_NO_FINAL = False
_WOUT_SPLIT = 2500  # DMA chunk width for W_out load (V = one DMA per k)


def _build(with_bout_mm: bool, zero_bias: bool = True):
    """Build + compile the per-core Bass program (identical across cores)."""
    nc = bacc.Bacc(
        "TRN2", target_bir_lowering=False, debug=False, num_devices=NCORES
    )

    featT = nc.dram_tensor("featT", [D, NS], BF16, kind="ExternalInput")
    tok_in = nc.dram_tensor("tok_in", [P, MT], I32, kind="ExternalInput")
    tok_out = nc.dram_tensor("tok_out", [P, MT], I32, kind="ExternalInput")
    w_out = nc.dram_tensor("w_out", [H, V], FP8, kind="ExternalInput")
    w_outT = nc.dram_tensor("w_outT", [V, AUG], BF16, kind="ExternalInput")
    w_embed = nc.dram_tensor("w_embed", [V, W], BF16, kind="ExternalInput")
    wh_d = nc.dram_tensor("wh", [H, H], BF16, kind="ExternalInput")
    wx_d = nc.dram_tensor("wx", [W, H], BF16, kind="ExternalInput")
    wproj_d = nc.dram_tensor("wproj", [D, H], BF16, kind="ExternalInput")
    b_d = nc.dram_tensor("b", [H, 1], F32, kind="ExternalInput")
    bp_d = nc.dram_tensor("b_proj", [H, 1], F32, kind="ExternalInput")
    bo_d = nc.dram_tensor("b_out_row", [1, V], F32, kind="ExternalInput")
    loss_d = nc.dram_tensor("loss", [1, 1], F32, kind="ExternalOutput")
    scratch_d = nc.dram_tensor("scratch", [P, 1], F32)  # internal
    if _DEBUG:
        dbg_partials = nc.dram_tensor("dbg_partials", [P, MT * NG], F32,
                                      kind="ExternalOutput")
        dbg_st = nc.dram_tensor("dbg_st", [P, MT], F32, kind="ExternalOutput")
        dbg_acc = nc.dram_tensor("dbg_acc", [P, MT], F32, kind="ExternalOutput")
        dbg_lse = nc.dram_tensor("dbg_lse", [P, MT], F32, kind="ExternalOutput")
        dbg_tot = nc.dram_tensor("dbg_tot", [P, 1], F32, kind="ExternalOutput")
        dbg_h = nc.dram_tensor("dbg_h", [P, MT * H], BF16, kind="ExternalOutput")

    with tile.TileContext(nc) as tc, ExitStack() as ctx:
        const = ctx.enter_context(tc.tile_pool(name="const", bufs=1))
        work = ctx.enter_context(tc.tile_pool(name="work", bufs=3))
        psum_sc = ctx.enter_context(tc.tile_pool(name="psc", bufs=2, space="PSUM"))
        psum_st = ctx.enter_context(tc.tile_pool(name="pst", bufs=1, space="PSUM"))
        psum_ms = ctx.enter_context(tc.tile_pool(name="pms", bufs=1, space="PSUM"))

        # ---- persistent SBUF tensors ----
        wout_sb = const.tile([P, KH * V], FP8)       # 40KB/part
        hT8 = const.tile([P, KH * R], FP8)           # fp8 copy of hT slots 1..64
        hT = const.tile([P, KH * HTB], BF16)         # 16.6KB/part
        aT = const.tile([P, KH * R], BF16)           # 16.4KB/part (x @ Wx + b)^T
        wg_sb = const.tile([P, MT * AUG], BF16)      # gathered target W_out rows
        h_rows = const.tile([P, MT * H], BF16)       # h row-major (DMA-transposed)
        xT_sb = const.tile([P, KW * R], BF16)
        wh_sb = const.tile([P, KH * KH * P], BF16)
        wx_sb = const.tile([P, KW * KH * P], BF16)
        tok_in_sb = const.tile([P, MT], I32)
        tok_out_sb = const.tile([P, MT], I32)
        b_sb = const.tile([P, KH], F32)
        bp_sb = const.tile([P, KH], F32)
        sumexp = const.tile([P, MT], F32)
        st_all = const.tile([P, MT], F32)
        lse = const.tile([P, MT], F32)
        nll = const.tile([P, MT], F32)
        mask = const.tile([P, MT], F32)
        tot = const.tile([P, 1], F32)
        tot_row = const.tile([1, P], F32)
        res = const.tile([1, 1], F32)
        warm = const.tile([P, 1], F32)
        if with_bout_mm:
            bo_sb = const.tile([1, V], F32)
            onesr = const.tile([1, P], F32)

        # ---- token / small-weight loads first: they gate the front-end
        # compute chain (gathers -> xT -> a -> recurrence).  W_out (10.2MB,
        # needed only once scores start) goes last, on the ScalarE HWDGE
        # queue so it doesn't head-of-line-block the SP queue.
        nc.sync.dma_start(tok_in_sb[:], tok_in[:, :])
        nc.sync.dma_start(tok_out_sb[:], tok_out[:, :])
        # biases: column k holds bias chunk for hidden block k
        nc.sync.dma_start(b_sb[:], bass.AP(b_d, 0, [[1, P], [P, KH]]))
        nc.sync.dma_start(bp_sb[:], bass.AP(bp_d, 0, [[1, P], [P, KH]]))
        # block layouts (k*KH+mp)*P are contiguous per k-chunk, so one DMA
        # per k row-slab; wproj/featT first (they gate h0 = PE's first work)
        wproj_sb = const.tile([P, KD * KH * P], BF16)
        featT_sb = const.tile([P, KD * NS], BF16)
        for k in range(KD):
            nc.sync.dma_start(wproj_sb[:, k * H:(k + 1) * H],
                              wproj_d[k * P:(k + 1) * P, :])
            nc.sync.dma_start(featT_sb[:, k * NS:(k + 1) * NS],
                              featT[k * P:(k + 1) * P, :])
        for k in range(KH):
            nc.sync.dma_start(wh_sb[:, k * H:(k + 1) * H],
                              wh_d[k * P:(k + 1) * P, :])
        for k in range(KW):
            nc.sync.dma_start(wx_sb[:, k * H:(k + 1) * H],
                              wx_d[k * P:(k + 1) * P, :])
        v0 = 0
        while v0 < V:
            wd = min(_WOUT_SPLIT, V - v0)
            for k in range(KH):
                nc.sync.dma_start(wout_sb[:, k * V + v0:k * V + v0 + wd],
                                  w_out[k * P:(k + 1) * P, v0:v0 + wd])
            v0 += wd
        if with_bout_mm:
            nc.sync.dma_start(bo_sb[:], bo_d[:, :])
            nc.gpsimd.memset(onesr[:], 1.0)

        # warm the ACT exp/tanh table set early (overlaps the big DMAs)
        nc.gpsimd.memset(warm[:], 0.0)
        nc.scalar.activation(warm[:], warm[:], AF.Exp)
        if _ABL_NO_SCORES:
            nc.gpsimd.memset(sumexp[:], 1.0)
        if _ABL_NO_ST:
            nc.gpsimd.memset(st_all[:], 0.0)
        if _ABL_NO_REC:
            nc.gpsimd.memset(hT[:], 0.0)

        ident = const.tile([P, P], BF16)
        from concourse.masks import make_identity
        make_identity(nc, ident[:])

        # ---- h0 first: PE's earliest work, gated only on wproj/featT ----
        if not _ABL_NO_PRE:
            for mp in range(KH):
                ps = psum_ms.tile([P, NS], F32, tag="ms")
                for k in range(KD):
                    nc.tensor.matmul(ps[:], lhsT=wproj_sb[:, (k * KH + mp) * P:
                                                          (k * KH + mp + 1) * P],
                                     rhs=featT_sb[:, k * NS:(k + 1) * NS],
                                     start=(k == 0), stop=(k == KD - 1))
                if zero_bias:
                    nc.vector.tensor_copy(hT[:, mp * HTB:mp * HTB + NS], ps[:])
                else:
                    nc.scalar.activation(hT[:, mp * HTB:mp * HTB + NS], ps[:],
                                         AF.Identity, bias=bp_sb[:, mp:mp + 1])

        # ---- gathers; xT via PE identity-transpose (PE is idle here and the
        # SP DMA queue would serialize 32 xbar transposes head-of-line).
        # Emitted per 512-row chunk, fused into the recurrence loop below:
        # chunk mc feeds exactly rec steps 4mc+1..4mc+4.
        xg_tiles = []
        if not _ABL_NO_GATHER:
            for mc in range(4):
                xg = const.tile([P, 4 * W], BF16, name=f"xg{mc}")
                xg_tiles.append(xg)
                for j in range(4):
                    m = mc * 4 + j
                    nc.gpsimd.indirect_dma_start(
                        out=xg[:, j * W:(j + 1) * W],
                        out_offset=None,
                        in_=w_embed[:, :],
                        in_offset=IndirectOffsetOnAxis(
                            ap=tok_in_sb[:, m:m + 1], axis=0),
                    )

        def emit_gather_chunk(mc):
            xg = xg_tiles[mc]
            for j in range(4):
                m = mc * 4 + j
                for k in range(KW):
                    # early in the kernel the scores PSUM pool is idle; use
                    # its double-buffered slots so transposes pipeline
                    pst_x = psum_sc.tile([P, P], BF16, tag="sc", name="pst_x")
                    nc.tensor.transpose(
                        pst_x[:], xg[:, j * W + k * P:j * W + (k + 1) * P],
                        ident[:])
                    nc.vector.tensor_copy(
                        xT_sb[:, k * R + m * P:k * R + (m + 1) * P], pst_x[:])
            # aT rows for this 512-row chunk immediately (rec step 1 needs
            # only chunk 0 -- don't make it wait for all 16 gathers)
            if not _ABL_NO_PRE:
                nch = mc
                for mp in range(KH):
                    ps = psum_ms.tile([P, 512], F32, tag="ms")
                    for k in range(KW):
                        nc.tensor.matmul(
                            ps[:],
                            lhsT=wx_sb[:, (k * KH + mp) * P:(k * KH + mp + 1) * P],
                            rhs=xT_sb[:, k * R + nch * 512:k * R + (nch + 1) * 512],
                            start=(k == 0), stop=(k == KW - 1))
                    if zero_bias:
                        nc.vector.tensor_copy(
                            aT[:, mp * R + nch * 512:mp * R + (nch + 1) * 512],
                            ps[:])
                    else:
                        nc.scalar.activation(
                            aT[:, mp * R + nch * 512:mp * R + (nch + 1) * 512],
                            ps[:], AF.Identity, bias=b_sb[:, mp:mp + 1])

        # target-column rows of W_out^T (+ b_out in col 512).  Emitted per-m
        # inside the main loop AFTER that chunk's x-gathers so the Pool queue
        # serves the ramp-critical x path first.
        def emit_wg_gather(m):
            nc.gpsimd.indirect_dma_start(
                out=wg_sb[:, m * AUG:(m + 1) * AUG],
                out_offset=None,
                in_=w_outT[:, :],
                in_offset=IndirectOffsetOnAxis(ap=tok_out_sb[:, m:m + 1], axis=0),
            )

        hT3 = hT[:].rearrange("p (b c) -> p b c", b=KH)
        aT3 = aT[:].rearrange("p (b c) -> p b c", b=KH)
        hT8_3 = hT8[:].rearrange("p (b c) -> p b c", b=KH)
        wout3 = wout_sb[:].rearrange("p (k c) -> p k c", k=KH)

        def emit_step(t):
            """h slot t (1..T) from slot t-1; PSUM [128, 4*NS], blocks = h'-chunks.
            The drive a_t enters via an identity matmul that opens the
            accumulation group, keeping the whole step chain on PE + ACT."""
            ps = psum_st.tile([P, KH * NS], F32, tag="step")
            nc.tensor.matmul(
                ps[:], lhsT=ident[:],
                rhs=aT3[:, :, (t - 1) * NS:t * NS],
                start=True, stop=False, skip_group_check=True)
            for mp in range(KH):
                for k in range(KH):
                    nc.tensor.matmul(
                        ps[:, mp * NS:(mp + 1) * NS],
                        lhsT=wh_sb[:, (k * KH + mp) * P:(k * KH + mp + 1) * P],
                        rhs=hT[:, k * HTB + (t - 1) * NS:k * HTB + t * NS],
                        start=False, stop=(mp == KH - 1 and k == KH - 1),
                        skip_group_check=True)
            ps3 = ps[:].rearrange("p (b n) -> p b n", b=KH)
            nc.scalar.activation(hT3[:, :, t * NS:(t + 1) * NS], ps3[:], AF.Tanh)
            # fp8 shadow of h_t for the DoubleRow score matmuls
            nc.vector.tensor_copy(hT8_3[:, :, (t - 1) * NS:t * NS],
                                  hT3[:, :, t * NS:(t + 1) * NS])

        # Per-row-tile exp accumulator: exp of group 0 writes it directly,
        # later groups exp into a scratch tile and are summed in on DVE
        # (bf16 tensor_tensor adds run in the DVE 2x perf mode; a direct
        # per-group tensor_reduce would run 1x and swamp DVE).  One final
        # 1x reduce per tile produces sum_v exp(s).  This keeps ACT free of
        # the per-instruction accum_out read (187ns each).
        acc_tiles = {}

        def emit_scores(m, groups):
            """vocab exp partial tiles for row tile m over the given groups."""
            for gi in groups:
                ps = psum_sc.tile([P, 1536], F32, tag="sc")
                off = 0
                for (voff, wd) in VGROUPS[gi]:
                    for g in range(KH // 2):
                        nc.tensor.matmul(
                            ps[:, off:off + wd],
                            lhsT=hT8_3[:, 2 * g:2 * g + 2, m * P:(m + 1) * P],
                            rhs=wout3[:, 2 * g:2 * g + 2, voff:voff + wd],
                            start=(g == 0),
                            stop=(g == KH // 2 - 1 and not with_bout_mm),
                            perf_mode=mybir.MatmulPerfMode.DoubleRow)
                    if with_bout_mm:
                        nc.tensor.matmul(
                            ps[:, off:off + wd],
                            lhsT=onesr[:, :],
                            rhs=bo_sb[:, voff:voff + wd],
                            start=False, stop=True,
                            skip_group_check=True)
                    off += wd
                if gi == 0:
                    acc = work.tile([P, 1536], BF16, tag="acc")
                    acc_tiles[m] = acc
                    nc.scalar.activation(acc[:, :off], ps[:, :off], AF.Exp)
                else:
                    esc = work.tile([P, 1536], BF16, tag="esc")
                    nc.scalar.activation(esc[:, :off], ps[:, :off], AF.Exp)
                    acc = acc_tiles[m]
                    nc.vector.tensor_add(acc[:, :off], acc[:, :off],
                                         esc[:, :off])

        def emit_reduce(m):
            acc = acc_tiles.pop(m)
            nc.vector.tensor_reduce(sumexp[:, m:m + 1], acc[:],
                                    axis=mybir.AxisListType.X, op=OP.add)

        def emit_hrows_st(m):
            for k in range(KH):
                nc.sync.dma_start_transpose(
                    h_rows[:, m * H + k * P:m * H + (k + 1) * P],
                    hT[:, k * HTB + NS + m * P:k * HTB + NS + (m + 1) * P])
            junk = work.tile([P, H], BF16, tag="junk")
            nc.vector.tensor_mul(junk[:], h_rows[:, m * H:(m + 1) * H],
                                 wg_sb[:, m * AUG:m * AUG + H])
            stp = work.tile([P, 1], F32, tag="stp")
            nc.vector.tensor_reduce(stp[:], junk[:],
                                    axis=mybir.AxisListType.X, op=OP.add)
            # + b_out[target] from the augmented gather column (zero here)
            nc.vector.tensor_add(st_all[:, m:m + 1], stp[:],
                                 wg_sb[:, m * AUG + H:m * AUG + H + 1])

        # ---- recurrence interleaved with scores of the previous row tile;
        # the first four iterations also run the gather->xT->aT pipeline ----
        group_chunks = [list(range(0, 2)), list(range(2, 4)),
                        list(range(4, 6)), list(range(6, NG))]
        for m in range(MT):
            if m < 4 and not _ABL_NO_GATHER:
                emit_gather_chunk(m)
            if not _ABL_NO_GATHER:
                emit_wg_gather(m)
            for j in range(4):
                if not _ABL_NO_REC:
                    emit_step(4 * m + j + 1)
                if m >= 1 and not _ABL_NO_SCORES:
                    emit_scores(m - 1, group_chunks[j])
            if not _ABL_NO_ST:
                emit_hrows_st(m)
            if m >= 1 and not _ABL_NO_SCORES:
                emit_reduce(m - 1)
        if not _ABL_NO_SCORES:
            emit_scores(MT - 1, list(range(NG)))
            emit_reduce(MT - 1)

        # ---- loss assembly ----
        nc.scalar.activation(lse[:], sumexp[:], AF.Ln)
        nc.vector.tensor_sub(nll[:], lse[:], st_all[:])
        nc.vector.tensor_scalar(mask[:], tok_out_sb[:], 0, None,
                                op0=OP.not_equal)
        junk2 = const.tile([P, MT], F32)
        nc.vector.tensor_mul(junk2[:], nll[:], mask[:])
        nc.vector.tensor_reduce(tot[:], junk2[:],
                                axis=mybir.AxisListType.X, op=OP.add)
        # cross-partition sum: bounce [128,1] -> DRAM -> [1,128], reduce
        nc.sync.dma_start(scratch_d[:, :], tot[:])
        nc.sync.dma_start(tot_row[:], bass.AP(scratch_d, 0, [[P, 1], [1, P]]))
        nc.vector.tensor_reduce(res[:], tot_row[:], axis=mybir.AxisListType.X,
                                op=OP.add)
        nc.sync.dma_start(loss_d[:, :], res[:])
        if _DEBUG:
            nc.sync.dma_start(dbg_st[:, :], st_all[:])
            nc.sync.dma_start(dbg_acc[:, :], sumexp[:])
            nc.sync.dma_start(dbg_lse[:, :], lse[:])
            nc.sync.dma_start(dbg_tot[:, :], tot[:])
            nc.sync.dma_start(dbg_h[:, :], h_rows[:])

    nc.compile()
    return nc


def _prepare_inputs(inputs):
    """Cast/shard host-side. Returns per-core in_maps."""
    feats = np.asarray(inputs["features"], dtype=np.float32)
    cap = np.asarray(inputs["captions"])
    W_proj = np.asarray(inputs["W_proj"], dtype=np.float32)
    b_proj = np.asarray(inputs["b_proj"], dtype=np.float32).reshape(H, 1)
    W_embed = np.asarray(inputs["W_embed"], dtype=np.float32)
    Wx = np.asarray(inputs["Wx"], dtype=np.float32)
    Wh = np.asarray(inputs["Wh"], dtype=np.float32)
    b = np.asarray(inputs["b"], dtype=np.float32).reshape(H, 1)
    W_out = np.asarray(inputs["W_out"], dtype=np.float32)
    b_out = np.asarray(inputs["b_out"], dtype=np.float32)

    bf = ml_dtypes.bfloat16
    f8 = ml_dtypes.float8_e4m3
    w_out_f8 = np.ascontiguousarray(W_out.astype(f8))
    w_embed_bf = np.ascontiguousarray(W_embed.astype(bf))
    wh_bf = np.ascontiguousarray(Wh.astype(bf))
    wx_bf = np.ascontiguousarray(Wx.astype(bf))
    wproj_bf = np.ascontiguousarray(W_proj.astype(bf))
    w_outT = np.zeros((V, AUG), dtype=bf)
    w_outT[:, :H] = W_out.T.astype(bf)
    w_outT[:, H] = b_out.astype(bf)
    bo_row = np.ascontiguousarray(b_out.reshape(1, V))

    shared = {
        "w_out": w_out_f8, "w_outT": w_outT, "w_embed": w_embed_bf,
        "wh": wh_bf, "wx": wx_bf, "wproj": wproj_bf,
        "b": b, "b_proj": b_proj, "b_out_row": bo_row,
    }
    in_maps = []
    for c in range(NCORES):
        rows = slice(c * NS, (c + 1) * NS)
        featT_c = np.ascontiguousarray(feats[rows].T.astype(bf))
        cin = np.asarray(cap[rows, :T], dtype=np.int32)     # [NS, T]
        cout = np.asarray(cap[rows, 1:T + 1], dtype=np.int32)
        # t-major flat r = t*NS + n, laid out as [128, MT] with r = m*128 + i
        tin = np.ascontiguousarray(cin.T.reshape(R).reshape(MT, P).T)
        tout = np.ascontiguousarray(cout.T.reshape(R).reshape(MT, P).T)
        in_maps.append({**shared, "featT": featT_c,
                        "tok_in": tin, "tok_out": tout})
    zero_bias = not (np.any(b) or np.any(b_proj))
    return in_maps, (bool(np.any(b_out != 0.0)), zero_bias)


def _get_program(flags=(False, True)):
    key = ("nc",) + tuple(flags)
    if key not in _CACHE:
        _CACHE[key] = _build(*flags)
    return _CACHE[key]


def kernel(**inputs) -> np.ndarray:
    in_maps, flags = _prepare_inputs(inputs)
    nc = _get_program(flags)
    out = run_bass_kernel_spmd(nc, in_maps, core_ids=list(range(NCORES)))
    total = sum(float(r["loss"][0, 0]) for r in out.results)
    return np.float32(total / N)



# revision 14
# speedup vs baseline: 1.2424x; 1.2214x over previous
"""CaptioningRNN forward loss on 8 Trainium2 NeuronCores.

Math (per reference):
    h0 = features @ W_proj + b_proj                       (no tanh)
    x  = W_embed[captions[:, :-1]]
    a  = x @ Wx + b                                       (precomputed input drive)
    h_t = tanh(h_{t-1} @ Wh + a_t)                        (T sequential steps)
    s  = h @ W_out + b_out                                (N*T x V logits)
    loss = sum over (n,t) of mask * (logsumexp(s) - s[target]) / N

Sharding: data-parallel over batch N=256 -> 32 rows/core, weights replicated.
Each core returns sumexp[r] = sum_v exp(s_rv) and st[r] = s_r,target; the
host finishes with sum(mask * (log(sumexp) - st)) / N (cheap glue: 16K logs).

On-chip (per core, t-major rows r = t*32 + n, tiled [128, m] with r=m*128+i):
  * xT produced directly by one dma_gather(transpose=True) pair from W_embed
    (token indices in the 16-partition-wrapped int16 layout); no PE
    transposes needed.
  * a = Wx.T-form matmul -> aT (bf16); chunk 0 split so the recurrence
    starts as early as possible, later chunks hidden inside the main loop.
  * recurrence in transposed form: hT[:, t] = tanh(Wh-blocks @ hT[:, t-1] + aT)
    one [128,128] PSUM tile per step (4 h'-chunks side by side in free dim).
  * scores: per 128-row tile, 7 vocab groups (6x1536 + 784); fp8 DoubleRow
    matmuls into PSUM, ACT Exp into a bf16 accumulator tile; groups 1..6
    summed in on DVE (2x-mode tensor_tensor adds) and one 1x tensor_reduce
    per row tile produces sumexp.  No accum_out -> no 187ns ACT reads.
  * emission order per window m: [sc(m-1,g0..g2), (step,sc g3), (step,g4),
    (step,g5), (step,g6)] so the in-order PE never head-of-line-blocks the
    exp stream on a tanh (each tanh unlocks the step+scores matmuls that
    feed the exp AFTER next, one full exp of slack).
  * target score: rows of W_out.T (+b_out col) fetched by one batched
    dma_gather per 4 row tiles; dot with h rows (DMA-transposed) on DVE.
"""

import sys

for _p in ("/opt/trn_rl_repo", "/root/.axon_site/_ro/trn_rl_repo"):
    if _p not in sys.path:
        sys.path.insert(0, _p)

import numpy as np
import ml_dtypes
from contextlib import ExitStack

import concourse.bass as bass
import concourse.tile as tile
from concourse import bacc, mybir
from concourse.bass_utils import run_bass_kernel_spmd

F32 = mybir.dt.float32
BF16 = mybir.dt.bfloat16
FP8 = mybir.dt.float8e4
I16 = mybir.dt.int16
AF = mybir.ActivationFunctionType
OP = mybir.AluOpType

# Problem sizes (hardcoded per spec).
N, T, D, W, H, V = 256, 64, 1280, 256, 512, 10000
NCORES = 8
NS = N // NCORES          # 32 batch rows per core
R = NS * T                # 2048 (t-major rows per core)
MT = R // 128             # 16 row tiles
KH = H // 128             # 4 hidden chunks
KW = W // 128             # 2 embed chunks
KD = D // 128             # 10 feature chunks
TSLOT = T + 1             # h slots (0 = h0)
HTB = TSLOT * NS          # 2080 columns per hidden-chunk block of hT
WGW_BO = 640              # gathered W_out^T row incl b_out col (256B mult)
WGW = 512                 # gathered W_out^T row, zero b_out (common case)
P = 128
NXG = 4                   # x-token gathers (512 idxs each; the HW
                          # transpose-gather ucode breaks above 512)
NWG = 4                   # target-token gathers (512 idxs each)

# vocab tiling: 512-wide matmuls, grouped into <=1536-wide exp groups
_VT = []
v = 0
while v < V:
    w = min(512, V - v)
    _VT.append((v, w))
    v += w
VGROUPS = []          # list of list[(voff, width)]
i = 0
while i < len(_VT):
    VGROUPS.append(_VT[i:i + 3])
    i += 3
NG = len(VGROUPS)     # 7 (6x1536 + 784)

_CACHE = {}
_WOUT_SPLIT = 2500  # DMA chunk width for W_out load


def _build(with_bout_mm: bool, zero_bias: bool = True):
    """Build + compile the per-core Bass program (identical across cores)."""
    nc = bacc.Bacc(
        "TRN2", target_bir_lowering=False, debug=False, num_devices=NCORES
    )

    featT = nc.dram_tensor("featT", [D, NS], FP8, kind="ExternalInput")
    tok16 = nc.dram_tensor("tok16", [P, 256], I16, kind="ExternalInput")
    w_out = nc.dram_tensor("w_out", [H, V], FP8, kind="ExternalInput")
    wgw = WGW_BO if with_bout_mm else WGW
    w_outT = nc.dram_tensor("w_outT", [V, wgw], BF16, kind="ExternalInput")
    w_embed = nc.dram_tensor("w_embed", [V, W], BF16, kind="ExternalInput")
    wh_d = nc.dram_tensor("wh", [H, H], BF16, kind="ExternalInput")
    wx_d = nc.dram_tensor("wx", [W, H], BF16, kind="ExternalInput")
    wproj_d = nc.dram_tensor("wproj", [D, H], FP8, kind="ExternalInput")
    b_d = nc.dram_tensor("b", [H, 1], F32, kind="ExternalInput")
    bp_d = nc.dram_tensor("b_proj", [H, 1], F32, kind="ExternalInput")
    bo_d = nc.dram_tensor("b_out_row", [1, V], F32, kind="ExternalInput")
    sumexp_d = nc.dram_tensor("sumexp", [P, MT], F32, kind="ExternalOutput")
    st_d = nc.dram_tensor("st", [P, MT], F32, kind="ExternalOutput")

    with tile.TileContext(nc) as tc, ExitStack() as ctx:
        const = ctx.enter_context(tc.tile_pool(name="const", bufs=1))
        work = ctx.enter_context(tc.tile_pool(name="work", bufs=3))
        psum_sc = ctx.enter_context(tc.tile_pool(name="psc", bufs=2, space="PSUM"))
        psum_st = ctx.enter_context(tc.tile_pool(name="pst", bufs=1, space="PSUM"))

        # ---- persistent SBUF tensors ----
        wout_sb = const.tile([P, KH * V], FP8)       # 40KB/part
        hT8 = const.tile([P, KH * R], FP8)           # fp8 copy of hT slots 1..64
        hT = const.tile([P, KH * HTB], BF16)         # 16.6KB/part
        wg_sb = const.tile([P, MT * wgw], BF16)      # gathered target W_out rows
        h_rows = const.tile([P, MT * H], BF16)       # h row-major (DMA-transposed)
        # xT in two tiles (one per gather: transposed gather output must be
        # a fully contiguous [128, KW, NI] block)
        xT_sb = [const.tile([P, KW * (R // NXG)], BF16, name=f"xT{g}")
                 for g in range(NXG)]
        wh_sb = const.tile([P, KH * KH * P], BF16)
        wx_sb = const.tile([P, KW * KH * P], BF16)
        wproj_sb = const.tile([P, KD * KH * P], FP8)
        featT_sb = const.tile([P, KD * NS], FP8)
        tok16_sb = const.tile([P, 256], I16)
        b_sb = const.tile([P, KH], F32)
        bp_sb = const.tile([P, KH], F32)
        sumexp = const.tile([P, MT], F32)
        st_all = const.tile([P, MT], F32)
        warm = const.tile([P, 1], F32)
        if with_bout_mm:
            bo_sb = const.tile([1, V], F32)
            onesr = const.tile([1, P], F32)

        # ---- DMAs: token indices first (they gate the gathers -> aT ->
        # recurrence chain), then small weights, then W_out (10MB) on the
        # ScalarE HWDGE queue.  Each weight is one DMA with a 3D access
        # pattern (k-chunk dim folded in) to cut HWDGE fixed overheads.
        # DMA priority: everything that gates the recurrence goes first on
        # the SP HWDGE queue (tok16 -> gathers; wh/wx -> steps; featT/wproj
        # (fp8) -> h0).  The 5.1MB W_out load rides the pool SWDGE queue
        # BEHIND the x gathers, so its 4x3.2us transfers can't starve the
        # ramp-critical path on the shared DMA engines; wg gathers go last
        # (their consumer, the target-score dot, also runs on Pool and is
        # only needed by the final output DMA).
        nc.sync.dma_start(tok16_sb[:], tok16[:, :])
        nc.sync.dma_start(featT_sb[:],
                          bass.AP(featT, 0, [[NS, P], [P * NS, KD], [1, NS]]))
        nc.sync.dma_start(wproj_sb[:],
                          bass.AP(wproj_d, 0, [[H, P], [P * H, KD], [1, H]]))
        nc.sync.dma_start(wx_sb[:], bass.AP(wx_d, 0, [[H, P], [P * H, KW], [1, H]]))
        nc.sync.dma_start(wh_sb[:], bass.AP(wh_d, 0, [[H, P], [P * H, KH], [1, H]]))
        if not zero_bias:
            nc.sync.dma_start(b_sb[:], bass.AP(b_d, 0, [[1, P], [P, KH]]))
            nc.sync.dma_start(bp_sb[:], bass.AP(bp_d, 0, [[1, P], [P, KH]]))
        if with_bout_mm:
            nc.scalar.dma_start(bo_sb[:], bo_d[:, :])
            nc.gpsimd.memset(onesr[:], 1.0)

        # ---- batched gathers (SWDGE): xT directly in transposed layout.
        # A 128-token pre-gather covers steps 1..4 so the recurrence can
        # start ~3us before the bulk gathers + W_out land. ----
        NI = R // NXG
        xTp = const.tile([P, KW * P], BF16)
        xTp3 = xTp[:].rearrange("p (k r) -> p k r", k=KW)
        nc.gpsimd.dma_gather(
            out_ap=xTp3[:, :, :], in_ap=w_embed[:, :],
            idxs_ap=tok16_sb[:, 0:8],
            num_idxs=P, num_idxs_reg=P, elem_size=W, transpose=True)
        xT3 = [t[:].rearrange("p (k r) -> p k r", k=KW) for t in xT_sb]
        for g in range(NXG):
            nc.gpsimd.dma_gather(
                out_ap=xT3[g][:, :, :],
                in_ap=w_embed[:, :],
                idxs_ap=tok16_sb[:, g * (NI // 16):(g + 1) * (NI // 16)],
                num_idxs=NI, num_idxs_reg=NI, elem_size=W, transpose=True)
        # W_out in fine-grained 1024-wide chunks: the shared DMA engines
        # serve requests FIFO, so small chunks interleave with (rather than
        # block) the ramp-critical gather/weight transfers, and chunk k
        # still lands roughly when the first exps need it (consumption
        # order matches vocab order)
        wout3 = wout_sb[:].rearrange("p (k c) -> p k c", k=KH)
        for ci, v0 in enumerate(range(0, V, 1024)):
            wd = min(1024, V - v0)
            inst = nc.sync.dma_start(
                wout3[:, :, v0:v0 + wd],
                bass.AP(w_out, v0, [[V, P], [P * V, KH], [1, wd]]))
            # explicitly AFTER the ramp-critical loads/gathers but well
            # before the exps that consume chunk ci (the scheduler would
            # otherwise greedily hoist these 1.5us transfers into the
            # entry-critical DMA window)
            inst.bass_priority = 150 + 25 * ci
        wg3 = wg_sb[:].rearrange("p (m e) -> p m e", e=wgw)
        NJ = R // NWG
        MB = MT // NWG
        for g in range(NWG):
            inst = nc.gpsimd.dma_gather(
                out_ap=wg3[:, g * MB:(g + 1) * MB, :],
                in_ap=w_outT[:, :],
                idxs_ap=tok16_sb[:, 128 + g * (NJ // 16):128 + (g + 1) * (NJ // 16)],
                num_idxs=NJ, num_idxs_reg=NJ, elem_size=wgw)
            inst.bass_priority = 800 + 10 * g

        # warm the ACT exp/tanh table set early (overlaps the big DMAs)
        nc.gpsimd.memset(warm[:], 0.0)
        nc.scalar.activation(warm[:], warm[:], AF.Exp)

        ident = const.tile([P, P], BF16)
        from concourse.masks import make_identity
        make_identity(nc, ident[:])
        # warm the PE out of its low p-state before the first real matmuls
        # (the cost model runs cold matmuls ~2-4x slower; a ~2us chain of
        # junk transposes brings the pipeline to full speed by ~4us)
        for _w in range(14):
            pw = psum_st.tile([P, P], BF16, tag="step")
            nc.tensor.transpose(pw[:], ident[:], ident[:])

        hT3 = hT[:].rearrange("p (b c) -> p b c", b=KH)
        hT8_3 = hT8[:].rearrange("p (b c) -> p b c", b=KH)

        # ---- h0 first: PE's earliest work, gated only on wproj/featT ----
        for mp in range(KH):
            ps = psum_st.tile([P, NS], F32, tag="step")
            for k in range(KD):
                nc.tensor.matmul(ps[:], lhsT=wproj_sb[:, (k * KH + mp) * P:
                                                      (k * KH + mp + 1) * P],
                                 rhs=featT_sb[:, k * NS:(k + 1) * NS],
                                 start=(k == 0), stop=(k == KD - 1))
            if zero_bias:
                nc.vector.tensor_copy(hT[:, mp * HTB:mp * HTB + NS], ps[:])
            else:
                nc.scalar.activation(hT[:, mp * HTB:mp * HTB + NS], ps[:],
                                     AF.Identity, bias=bp_sb[:, mp:mp + 1])

        def emit_step(t):
            """h slot t (1..T) from slot t-1; PSUM [128, 4*NS], blocks =
            h'-chunks.  The input drive Wx @ x_t is contracted directly in
            the same accumulation group as Wh @ h_{t-1} (no precomputed aT
            tile, no PSUM-pool contention for it).  Steps 1..4 read x from
            the 128-token pre-gather so the recurrence starts before the
            bulk gathers land."""
            ps = psum_st.tile([P, KH * NS], F32, tag="step")
            c0 = (t - 1) * NS
            for mp in range(KH):
                for k in range(KW):
                    x3 = xTp3 if t <= 4 else xT3[c0 // NI]
                    nc.tensor.matmul(
                        ps[:, mp * NS:(mp + 1) * NS],
                        lhsT=wx_sb[:, (k * KH + mp) * P:(k * KH + mp + 1) * P],
                        rhs=x3[:, k, c0 % NI if t > 4 else c0:
                               (c0 % NI if t > 4 else c0) + NS],
                        start=(k == 0), stop=False, skip_group_check=True)
                for k in range(KH):
                    nc.tensor.matmul(
                        ps[:, mp * NS:(mp + 1) * NS],
                        lhsT=wh_sb[:, (k * KH + mp) * P:(k * KH + mp + 1) * P],
                        rhs=hT[:, k * HTB + (t - 1) * NS:k * HTB + t * NS],
                        start=False, stop=(mp == KH - 1 and k == KH - 1),
                        skip_group_check=True)
            ps3 = ps[:].rearrange("p (b n) -> p b n", b=KH)
            if zero_bias:
                nc.scalar.activation(hT3[:, :, t * NS:(t + 1) * NS], ps3[:],
                                     AF.Tanh)
            else:
                for mp in range(KH):
                    nc.scalar.activation(
                        hT3[:, mp, t * NS:(t + 1) * NS],
                        ps3[:, mp, :], AF.Tanh, bias=b_sb[:, mp:mp + 1])
            # fp8 shadow of h_t for the DoubleRow score matmuls
            nc.vector.tensor_copy(hT8_3[:, :, (t - 1) * NS:t * NS],
                                  hT3[:, :, t * NS:(t + 1) * NS])

        # Per-row-tile exp accumulator: exp of group 0 writes it directly,
        # later groups exp into a scratch tile and are summed in on DVE
        # (bf16 tensor_tensor adds run in the DVE 2x perf mode).  One final
        # 1x tensor_reduce per tile produces sum_v exp(s).  This keeps ACT
        # free of per-instruction accum_out reads.
        acc_tiles = {}
        strip = const.tile([P, NG], F32)   # last-tile accum_out partials

        def emit_scores(m, gi):
            ps = psum_sc.tile([P, 1536], F32, tag="sc")
            off = 0
            for (voff, wd) in VGROUPS[gi]:
                for g in range(KH // 2):
                    nc.tensor.matmul(
                        ps[:, off:off + wd],
                        lhsT=hT8_3[:, 2 * g:2 * g + 2, m * P:(m + 1) * P],
                        rhs=wout3[:, 2 * g:2 * g + 2, voff:voff + wd],
                        start=(g == 0),
                        stop=(g == KH // 2 - 1 and not with_bout_mm),
                        perf_mode=mybir.MatmulPerfMode.DoubleRow)
                if with_bout_mm:
                    nc.tensor.matmul(
                        ps[:, off:off + wd],
                        lhsT=onesr[:, :],
                        rhs=bo_sb[:, voff:voff + wd],
                        start=False, stop=True,
                        skip_group_check=True)
                off += wd
            if m == MT - 1:
                # last tile: ACT accum_out (187ns aux reads) instead of DVE
                # adds -- the tail chain shrinks to one tiny 7-wide reduce
                esc = work.tile([P, 1536], BF16, tag="esc")
                nc.scalar.activation(esc[:, :off], ps[:, :off], AF.Exp,
                                     accum_out=strip[:, gi:gi + 1])
            elif gi == 0:
                acc = work.tile([P, 1536], BF16, tag="acc")
                acc_tiles[m] = acc
                nc.scalar.activation(acc[:, :off], ps[:, :off], AF.Exp)
            else:
                esc = work.tile([P, 1536], BF16, tag="esc")
                nc.scalar.activation(esc[:, :off], ps[:, :off], AF.Exp)
                acc = acc_tiles[m]
                nc.vector.tensor_add(acc[:, :off], acc[:, :off], esc[:, :off])

        def emit_reduce(m):
            if m == MT - 1:
                nc.vector.tensor_reduce(sumexp[:, m:m + 1], strip[:],
                                        axis=mybir.AxisListType.X, op=OP.add)
                return
            acc = acc_tiles.pop(m)
            nc.vector.tensor_reduce(sumexp[:, m:m + 1], acc[:],
                                    axis=mybir.AxisListType.X, op=OP.add)

        def emit_hrows(m):
            for k in range(KH):
                nc.sync.dma_start_transpose(
                    h_rows[:, m * H + k * P:m * H + (k + 1) * P],
                    hT[:, k * HTB + NS + m * P:k * HTB + NS + (m + 1) * P])

        def emit_st(m):
            """Target-score dot for tile m.  Runs 4 windows behind the
            h_rows transpose so the wg gather (last in the DMA priority
            order) can never head-of-line-block the DVE queue."""
            junk = work.tile([P, H], BF16, tag="junk")
            nc.vector.tensor_mul(junk[:], h_rows[:, m * H:(m + 1) * H],
                                 wg_sb[:, m * wgw:m * wgw + H])
            if with_bout_mm:
                stp = work.tile([P, 1], F32, tag="stp")
                nc.vector.tensor_reduce(stp[:], junk[:],
                                        axis=mybir.AxisListType.X, op=OP.add)
                # + b_out[target] from the augmented gather column
                nc.vector.tensor_add(st_all[:, m:m + 1], stp[:],
                                     wg_sb[:, m * wgw + H:m * wgw + H + 1])
            else:
                nc.vector.tensor_reduce(st_all[:, m:m + 1], junk[:],
                                        axis=mybir.AxisListType.X, op=OP.add)

        # ---- main loop.  Window m runs steps 4m+1..4m+4 and the scores of
        # row tile m-2: the two-window lag means every score matmul's hT8
        # inputs were finished a full window earlier, so the in-order PE
        # never stalls the exp stream on a tanh.  aT chunk c lands in
        # window c (chunk 0 pre-split before the loop so step 1 starts
        # early). ----
        # Flat (tile, group) scores work list consumed by a cursor: window
        # m may emit items of tiles <= m-1 (their steps finished a window
        # earlier).  Window 1 takes 4 items (fills the otherwise-idle ACT
        # during the early recurrence, rate-matched to the W_out chunk
        # arrivals); later windows take NG items, so the scores stream runs
        # ~1.5 windows behind the recurrence and the trailing tail after
        # the last step is only ~10 items.
        witems = [(t, g) for t in range(MT) for g in range(NG)]
        cursor = 0

        def emit_items(n):
            nonlocal cursor
            for (t, g) in witems[cursor:cursor + n]:
                emit_scores(t, g)
                if g == NG - 1:
                    emit_reduce(t)
            cursor += n

        for m in range(MT):
            want = 0 if m <= 1 else NG
            take = max(0, min(want, m * NG - cursor))
            q, r = divmod(take, 4)
            split = [q + (1 if j < r else 0) for j in range(4)]
            for j in range(4):
                emit_step(4 * m + j + 1)
                emit_items(split[j])
            emit_hrows(m)
            if m >= 4:
                emit_st(m - 4)
        emit_items(len(witems) - cursor)
        for mm_ in range(MT - 4, MT):
            emit_st(mm_)

        # ---- ship per-row sumexp and target scores; host does
        # mask * (log(sumexp) - st) ----
        nc.sync.dma_start(sumexp_d[:, :], sumexp[:])
        nc.sync.dma_start(st_d[:, :], st_all[:])

    nc.compile()
    return nc


def _wrap16(flat: np.ndarray, nblk: int) -> np.ndarray:
    """Wrap a flat int index list into the dma_gather layout: token f of
    block g at [f%16, g*cols + f//16], with the 16-partition block
    replicated down all 128 partitions (each DMA-engine group reads its
    own copy -- unreplicated rows silently gather row 0 on HW)."""
    n = flat.shape[0] // nblk
    cols = n // 16
    out = np.zeros((16, nblk * cols), dtype=np.int16)
    for g in range(nblk):
        out[:, g * cols:(g + 1) * cols] = flat[g * n:(g + 1) * n].reshape(cols, 16).T
    return np.tile(out, (8, 1))


def _prepare_inputs(inputs):
    """Cast/shard host-side. Returns per-core in_maps plus host-side mask."""
    feats = np.asarray(inputs["features"], dtype=np.float32)
    cap = np.asarray(inputs["captions"])
    W_proj = np.asarray(inputs["W_proj"], dtype=np.float32)
    b_proj = np.asarray(inputs["b_proj"], dtype=np.float32).reshape(H, 1)
    W_embed = np.asarray(inputs["W_embed"], dtype=np.float32)
    Wx = np.asarray(inputs["Wx"], dtype=np.float32)
    Wh = np.asarray(inputs["Wh"], dtype=np.float32)
    b = np.asarray(inputs["b"], dtype=np.float32).reshape(H, 1)
    W_out = np.asarray(inputs["W_out"], dtype=np.float32)
    b_out = np.asarray(inputs["b_out"], dtype=np.float32)

    bf = ml_dtypes.bfloat16
    f8 = ml_dtypes.float8_e4m3
    w_out_f8 = np.ascontiguousarray(W_out.astype(f8))
    w_embed_bf = np.ascontiguousarray(W_embed.astype(bf))
    wh_bf = np.ascontiguousarray(Wh.astype(bf))
    wx_bf = np.ascontiguousarray(Wx.astype(bf))
    wproj_f8 = np.ascontiguousarray(W_proj.astype(f8))
    wgw = WGW_BO if np.any(b_out != 0.0) else WGW
    w_outT = np.zeros((V, wgw), dtype=bf)
    w_outT[:, :H] = W_out.T.astype(bf)
    if wgw == WGW_BO:
        w_outT[:, H] = b_out.astype(bf)
    bo_row = np.ascontiguousarray(b_out.reshape(1, V))

    shared = {
        "w_out": w_out_f8, "w_outT": w_outT, "w_embed": w_embed_bf,
        "wh": wh_bf, "wx": wx_bf, "wproj": wproj_f8,
        "b": b, "b_proj": b_proj, "b_out_row": bo_row,
    }
    in_maps = []
    masks = []
    for c in range(NCORES):
        rows = slice(c * NS, (c + 1) * NS)
        featT_c = np.ascontiguousarray(feats[rows].T.astype(f8))
        cin = np.asarray(cap[rows, :T], dtype=np.int32)     # [NS, T]
        cout = np.asarray(cap[rows, 1:T + 1], dtype=np.int32)
        # t-major flat r = t*NS + n
        fin = cin.T.reshape(R)
        fout = cout.T.reshape(R)
        tok16 = np.zeros((P, 256), dtype=np.int16)
        tok16[:, :128] = _wrap16(fin, NXG)
        tok16[:, 128:] = _wrap16(fout, NWG)
        # [128, MT] mask in the same layout the kernel writes sumexp/st
        tout = np.ascontiguousarray(fout.reshape(MT, P).T)
        masks.append((tout != 0).astype(np.float64))
        in_maps.append({**shared, "featT": featT_c, "tok16": tok16})
    zero_bias = not (np.any(b) or np.any(b_proj))
    return in_maps, masks, (bool(np.any(b_out != 0.0)), zero_bias)


def _get_program(flags=(False, True)):
    key = ("nc",) + tuple(flags)
    if key not in _CACHE:
        _CACHE[key] = _build(*flags)
    return _CACHE[key]


def kernel(**inputs) -> np.ndarray:
    in_maps, masks, flags = _prepare_inputs(inputs)
    nc = _get_program(flags)
    out = run_bass_kernel_spmd(nc, in_maps, core_ids=list(range(NCORES)))
    total = 0.0
    for c, r in enumerate(out.results):
        se = np.asarray(r["sumexp"], dtype=np.float64)
        st = np.asarray(r["st"], dtype=np.float64)
        total += float((masks[c] * (np.log(se) - st)).sum())
    return np.float32(total / N)
